# revision 8
# baseline (speedup 1.0000x reference)
"""MoE layer (dense all-expert routing) Trainium2 Bass kernel.

Problem: x[4,2048,1024] f32, gate_w[1024,8], gate_b[8], expert_w[8,1024,1024].
  gate = softmax(x @ gate_w + gate_b)                  # [B,S,E]
  out  = einsum('bse,bseo->bso', gate, einsum('bsi,eio->bseo', x, expert_w))

Sharding: data-parallel over tokens. 8192 tokens split into 8 shards of 1024;
each core computes its shard against all 8 experts (weights replicated).
No collectives; host concatenates shard outputs.

Per-core kernel (strategy: fused gate-scaled PSUM combine):
  - xT (d_in-major) and all expert weights streamed as bf16; PSUM accumulates f32.
  - Gate logits computed on PE in [token, expert] orientation so softmax is
    all free-dim ops; bias added via a partition-broadcast DMA of gate_b.
  - Main loop: for each (n-half, expert, k-tile) stream W tile, matmul into
    per-m PSUM tiles (accumulate over k); per expert, fold into SBUF
    accumulator with one fused DVE op: acc = (psum * g[:,e]) + acc.
"""

import numpy as np
import ml_dtypes
from contextlib import ExitStack

import concourse.bacc as bacc
import concourse.bass as bass
import concourse.mybir as mybir
import concourse.tile as tile

BF16 = mybir.dt.bfloat16
F32 = mybir.dt.float32

P = 128  # partitions


def build_moe_nc(T=1024, D=1024, O=1024, E=8, NO=512, w_bufs=6, acc_bufs=16):
    """Build the per-core Bass program.

    T: tokens per core, D: d_in, O: d_out, E: experts, NO: d_out tile (<=512).
    """
    KT = D // P   # k tiles (contraction)
    MT = T // P   # token tiles
    NT = O // NO  # d_out tiles

    nc = bacc.Bacc("TRN2", target_bir_lowering=False, debug=False)
    xT_d = nc.dram_tensor("xT", [D, T], BF16, kind="ExternalInput")
    w_d = nc.dram_tensor("w", [E, D, O], BF16, kind="ExternalInput")
    gw_d = nc.dram_tensor("gw", [D, E], BF16, kind="ExternalInput")
    gb_d = nc.dram_tensor("gb", [1, E], F32, kind="ExternalInput")
    out_d = nc.dram_tensor("out", [T, O], F32, kind="ExternalOutput")

    with tile.TileContext(nc) as tc:
        with ExitStack() as ctx:
            singles = ctx.enter_context(tc.tile_pool(name="singles", bufs=1))
            wpool = ctx.enter_context(tc.tile_pool(name="w", bufs=w_bufs))
            accp = ctx.enter_context(tc.tile_pool(name="acc", bufs=acc_bufs))
            gpool = ctx.enter_context(tc.tile_pool(name="gate", bufs=1))
            ps = ctx.enter_context(
                tc.tile_pool(name="ps", bufs=8, space="PSUM")
            )

            # ---- resident loads -------------------------------------------
            xT = []
            for k in range(KT):
                t = singles.tile([P, T], BF16, tag=f"xT{k}")
                nc.sync.dma_start(out=t, in_=xT_d[k * P:(k + 1) * P, :])
                xT.append(t)
            gw = []
            for k in range(KT):
                t = singles.tile([P, E], BF16, tag=f"gw{k}")
                nc.sync.dma_start(out=t, in_=gw_d[k * P:(k + 1) * P, :])
                gw.append(t)
            gb_b = singles.tile([P, E], F32, tag="gb")
            nc.sync.dma_start(out=gb_b, in_=gb_d[0:1, :].to_broadcast([P, E]))

            # ---- gate: logits -> softmax, [token, expert] orientation -----
            g_sb = []  # per m-tile: [P, E] f32 normalized gate weights
            for m in range(MT):
                psg = ps.tile([P, NO], F32, tag="ps")
                for k in range(KT):
                    nc.tensor.matmul(
                        psg[:, 0:E],
                        lhsT=xT[k][:, m * P:(m + 1) * P],
                        rhs=gw[k],
                        start=(k == 0),
                        stop=(k == KT - 1),
                    )
                lg = gpool.tile([P, E], F32, tag=f"lg{m}")
                nc.vector.tensor_add(lg, psg[:, 0:E], gb_b)
                p_t = gpool.tile([P, E], F32, tag=f"p{m}")
                s_t = gpool.tile([P, 1], F32, tag=f"s{m}")
                # exp(logits); |logits| <~ 3 so no max-subtraction needed
                nc.scalar.activation(
                    p_t, lg, mybir.ActivationFunctionType.Exp, accum_out=s_t
                )
                rs_t = gpool.tile([P, 1], F32, tag=f"rs{m}")
                nc.vector.reciprocal(rs_t, s_t)
                g_t = gpool.tile([P, E], F32, tag=f"g{m}")
                nc.vector.tensor_scalar_mul(g_t, p_t, rs_t)
                g_sb.append(g_t)

            # ---- main: all-expert GEMM + fused gate combine ---------------
            for n in range(NT):
                acc = [None] * MT
                for e in range(E):
                    psy = [None] * MT
                    for k in range(KT):
                        wt = wpool.tile([P, NO], BF16, tag="w")
                        nc.sync.dma_start(
                            out=wt,
                            in_=w_d[e, k * P:(k + 1) * P, n * NO:(n + 1) * NO],
                        )
                        for m in range(MT):
                            if k == 0:
                                psy[m] = ps.tile(
                                    [P, NO], F32, tag="ps", name=f"psy{m}"
                                )
                            nc.tensor.matmul(
                                psy[m],
                                lhsT=xT[k][:, m * P:(m + 1) * P],
                                rhs=wt,
                                start=(k == 0),
                                stop=(k == KT - 1),
                            )
                    for m in range(MT):
                        if e == 0:
                            acc[m] = accp.tile(
                                [P, NO], F32, tag="acc", name=f"acc{m}"
                            )
                            nc.vector.tensor_scalar_mul(
                                acc[m], psy[m], g_sb[m][:, 0:1]
                            )
                        else:
                            nc.vector.scalar_tensor_tensor(
                                out=acc[m],
                                in0=psy[m],
                                scalar=g_sb[m][:, e:e + 1],
                                in1=acc[m],
                                op0=mybir.AluOpType.mult,
                                op1=mybir.AluOpType.add,
                            )
                for m in range(MT):
                    nc.sync.dma_start(
                        out=out_d[m * P:(m + 1) * P, n * NO:(n + 1) * NO],
                        in_=acc[m],
                    )
    nc.compile()
    return nc


# ---------------------------------------------------------------------------
# Host wrapper: full inputs -> shard -> run SPMD on 8 cores -> gather
# ---------------------------------------------------------------------------

N_CORES = 8
_B, _S, _DIN, _DOUT, _E = 4, 2048, 1024, 1024, 8


LAST_RESULTS = None  # BassKernelResults of the most recent run (for profiling)


def kernel(x, gate_w, gate_b, expert_w, _trace=False):
    global LAST_RESULTS
    from concourse.bass_utils import run_bass_kernel_spmd

    x = np.asarray(x)
    tokens = x.reshape(-1, _DIN)  # [8192, 1024]
    n_tok = tokens.shape[0]
    tpc = n_tok // N_CORES  # tokens per core

    w_bf = np.asarray(expert_w, dtype=ml_dtypes.bfloat16)
    gw_bf = np.asarray(gate_w, dtype=ml_dtypes.bfloat16)
    gb_f = np.asarray(gate_b, dtype=np.float32).reshape(1, _E)

    in_maps = []
    for c in range(N_CORES):
        shard = tokens[c * tpc:(c + 1) * tpc]  # [1024, 1024]
        xT = np.ascontiguousarray(shard.T).astype(ml_dtypes.bfloat16)
        in_maps.append({"xT": xT, "w": w_bf, "gw": gw_bf, "gb": gb_f})

    nc = build_moe_nc(T=tpc, D=_DIN, O=_DOUT, E=_E)
    res = run_bass_kernel_spmd(nc, in_maps, list(range(N_CORES)), trace=_trace)
    LAST_RESULTS = res
    outs = [res.results[c]["out"] for c in range(N_CORES)]
    full = np.concatenate(outs, axis=0).astype(np.float32)
    return full.reshape(_B, _S, _DOUT)
